# revision 33
# baseline (speedup 1.0000x reference)
"""SupCon cluster-memory loss kernel for 8 TRN2 NeuronCores — raw bass.

Math (per core, N-shard of 1024 bank rows x 4 (anchor, bank) combos):
  sumexp[i] = sum_j exp((x_a . mem_b_j)/T - shift_b)
via fp8 DoubleRow matmuls + ScalarE Exp + VectorE/GpSimd row-sums.
The positives term is host-side index bookkeeping (no device work).

v2 schedule (vs the whole-block baseline):
- All input DMAs are kp-sliced (<=131KB).  Probe-measured HWDGE behavior:
  a <=131KB transfer's completion semaphore lands WITH the data; the
  2.3us final-increment lag only afflicts large back-to-back transfers.
  So the matmul stream starts ~3.5us earlier.
- Three queues: scalar + sync HWDGE rings carry bank-1 slices, x slices
  and shift (consumed first); the gpsimd SWDGE ring carries all bank-0
  slices (needed ~8us later; SWDGE's ~2.6us startup is irrelevant).
- Supergroups de-interleaved (one (b,mt,a) at a time) so sg1 completes
  as early as DMA allows; sg1's exp is split into nt-halves so the
  Scalar engine starts exping at ~10.5us.
- Exp chunks: sg1 halves, sg2..sg7 full, sg8 halves (last half with
  fused row-accum into rs).  Row-reduces are split across Vector and
  GpSimd so no engine backlogs at stream end.
- No trailing drains: the neuronxcc custom-kernel lowering appends its
  own all-engine barrier + drains + a fixed ~6.5us sweep zeroing sems
  3..255 on every execution.
"""

import ml_dtypes
import numpy as np

import concourse.bacc as bacc
import concourse.mybir as mybir
from concourse.bass_utils import run_bass_kernel_spmd

BF16_NP = ml_dtypes.bfloat16

B = 256          # anchor batch per modality
N = 8192         # memory bank rows
D = 768          # feature dim
NCORES = 8
NS = N // NCORES     # 1024 bank rows per core
KT = D // 128        # 6 contraction tiles
MT = B // 128        # 2 anchor partition tiles
SUPCON_T = 0.07

F32 = mybir.dt.float32
FP8 = mybir.dt.float8e4
FP8_NP = ml_dtypes.float8_e4m3
FP8_SCALE = 16.0

NWU = 24         # warmup matmuls: ~4.3us of continuous PE work so the
                 # HAM clock ramp reaches 2.4GHz before the real stream
NOUT = 16        # rs cols: one per sg-half (see CHUNKS)

# Supergroups (b, mt, a); psum bank = si % 4 (b=1 sgs then b=0 reuse).
SGS = [(1, 0, 0), (1, 1, 0), (1, 0, 1), (1, 1, 1),
       (0, 0, 0), (0, 1, 0), (0, 0, 1), (0, 1, 1)]
SG_ACC = [0, 1, 2, 3, 0, 1, 2, 3]

# Exp chunks in s_mm order: (si_list, nt, rs_col_start, owner).
# b1 chunks are strided [128, 2, 512] pair-ACTIVATEs over adjacent psum
# banks — valid because the logsumexp shift is a per-bank constant
# (normalized prototypes; see make_in_maps).  The b0 chunks are singles
# (sg-serial stream order) so the Scalar engine's b0 exp crunch starts
# as early as possible; the last two use the fused ACT row-accumulator.
CHUNKS = [([0, 1], 0, 0, 'v'), ([2, 3], 0, 2, 'v'),
          ([0, 1], 1, 4, 'v'), ([2, 3], 1, 6, 'v'),
          ([4], 0, 8, 'v'), ([5], 0, 9, 'v'),
          ([6], 0, 10, 'v'), ([7], 0, 11, 'v'),
          ([4, 5], 1, 12, 'v'),
          ([6], 1, 14, 'a'), ([7], 1, 15, 'a')]

_NC_CACHE = {}


def _build_nc():
    nc = bacc.Bacc("TRN2", target_bir_lowering=False, debug=False,
                   num_devices=NCORES)

    # xT split by anchor half a: [2][128, KT, B] fp8.
    xT = nc.dram_tensor("xT", [2, 128, KT, B], FP8, kind="ExternalInput").ap()
    # memB[b][nt] = [128, KT, 512] fp8 column block of bank b.
    memB = nc.dram_tensor("memB", [2, 2, 128, KT, 512], FP8,
                          kind="ExternalInput").ap()
    nshift_h = nc.dram_tensor("nshift", [128, MT, 2, 2], F32,
                              kind="ExternalInput").ap()
    res = nc.dram_tensor("res", [128, NOUT], F32, kind="ExternalOutput").ap()

    x_sb = [nc.alloc_sbuf_tensor(f"x{a}", [128, KT, B], FP8).ap()
            for a in range(2)]
    blk = {(b, nt): nc.alloc_sbuf_tensor(f"m{b}{nt}", [128, KT, 512], FP8).ap()
           for b in range(2) for nt in range(2)}
    shift_t = nc.alloc_sbuf_tensor("shift", [128, MT, 2, 2], F32).ap()
    rs = nc.alloc_sbuf_tensor("rs", [128, NOUT], F32).ap()
    wu_w = nc.alloc_sbuf_tensor("wu_w", [128, 128], FP8).ap()
    wu_r = nc.alloc_sbuf_tensor("wu_r", [128, 256], FP8).ap()
    acc_all = nc.alloc_psum_tensor("acc", [128, 4, 1024], F32).ap()

    def sem(name):
        return nc.alloc_semaphore(name)

    # ---- DMA slice plan.  Each entry: (engine, kind, args) in queue
    # order; kind 'b' = bank slice (b, nt, kp), 'x' = x slice (a, kp),
    # 'xw' = x whole-remainder, 's' = shift.
    # Only the b1/x0 front needs tight per-slice completion sems; it is
    # kp-sliced across both HWDGE rings in consumption order.  The b0
    # banks ship as whole 393KB blocks ('B'): better queue throughput,
    # and their (laggier) completions still land 2-4us before use.
    # x1/shift ride the gpsimd SWDGE.
    q_scalar = [('x', 0, 0), ('b', 1, 0, 0), ('B', 0, 0), ('B', 0, 1)]
    q_sync = [('x', 0, 1), ('b', 1, 0, 1), ('x', 0, 2), ('b', 1, 1, 0),
              ('b', 1, 1, 1), ('b', 1, 1, 2)]
    q_gp = [('b', 1, 0, 2), ('x', 1, 0), ('s',), ('x', 1, 1), ('x', 1, 2)]

    bank_sem = {}
    x_sem = {}
    shift_sem = None

    def issue(eng, plan):
        nonlocal shift_sem
        for item in plan:
            if item[0] == 'b':
                _, b, nt, kp = item
                s = sem(f"d_b{b}{nt}{kp}")
                eng.dma_start(
                    out=blk[b, nt][:, 2 * kp:2 * kp + 2],
                    in_=memB[b, nt][:, 2 * kp:2 * kp + 2]).then_inc(s, 16)
                bank_sem[(b, nt, kp)] = s
            elif item[0] == 'B':
                _, b, nt = item
                s = sem(f"d_B{b}{nt}")
                eng.dma_start(out=blk[b, nt],
                              in_=memB[b, nt]).then_inc(s, 16)
                for kp in range(KT // 2):
                    bank_sem[(b, nt, kp)] = s
            elif item[0] == 'x':
                _, a, kp = item
                s = sem(f"d_x{a}{kp}")
                eng.dma_start(
                    out=x_sb[a][:, 2 * kp:2 * kp + 2],
                    in_=xT[a][:, 2 * kp:2 * kp + 2]).then_inc(s, 16)
                x_sem[(a, kp)] = s
            else:
                s = sem("d_shift")
                eng.dma_start(out=shift_t, in_=nshift_h).then_inc(s, 16)
                shift_sem = s

    issue(nc.scalar, q_scalar)
    issue(nc.sync, q_sync)
    issue(nc.gpsimd, q_gp)

    s_mm = sem("s_mm")      # chunk accumulation done (Tensor), CHUNKS order
    s_exp = sem("s_exp")    # chunk exp done (Scalar)
    s_redv = sem("s_redv")  # Vector reduces done
    s_redg = sem("s_redg")  # GpSimd reduces done
    s_acc = sem("s_acc")    # final accum chunk retired (Scalar)
    s_fin = sem("s_fin")    # output DMA

    # ---- Tensor: warmups then the 48-matmul stream, kp-quad-major:
    # each bank slice feeds four matmuls (one per sg of the phase), so
    # ~0.86us of PE work per 131KB slice matches the 2-queue fill rate.
    for _ in range(NWU):
        nc.tensor.matmul(acc_all[:, 0, 0:256], wu_w, wu_r,
                         start=True, stop=True)

    hi = {}

    def twait(s, v):
        if hi.get(s.num, 0) < v:
            hi[s.num] = v
            nc.tensor.wait_ge(s, v)

    def task(si, nt, kp):
        b, mt, a = SGS[si]
        twait(bank_sem[(b, nt, kp)], 16)
        twait(x_sem[(a, kp)], 16)
        return nc.tensor.matmul(
            acc_all[:, SG_ACC[si], nt * 512:(nt + 1) * 512],
            x_sb[a][:, 2 * kp:2 * kp + 2, mt * 128:(mt + 1) * 128],
            blk[b, nt][:, 2 * kp:2 * kp + 2],
            start=(kp == 0), stop=(kp == KT // 2 - 1),
            perf_mode=mybir.MatmulPerfMode.DoubleRow)

    KL = KT // 2
    # b1 phases: kp-quad-major (supply-matched to the sliced fill).
    # Chunk sems (s_mm) fire on the pair's later sg at kp2.
    for nt in range(2):
        for kp in range(KL):
            for si in range(4):
                mm = task(si, nt, kp)
                if kp == KL - 1 and si in (1, 3):
                    mm.then_inc(s_mm, 1)
    # b0 phases: whole blocks arrive by ~13us, so order sg-serial for the
    # earliest possible per-sg chunk completions.  WAR: sg si reuses the
    # psum region of sg si-4, freed by that chunk's Vector reduce.
    for nt in range(2):
        for si in range(4, 8):
            # chunk index of the donor (b1) chunk in s_redv counting:
            # nt0: c1 (banks 0-1) / c2 (banks 2-3); nt1: c3 / c4.
            twait(s_redv, 2 * nt + 1 + (si >= 6))
            for kp in range(KL):
                mm = task(si, nt, kp)
            # b0-nt1: si4/si5 share a pair chunk ending at si5.
            if not (nt == 1 and si == 4):
                mm.then_inc(s_mm, 1)

    # ---- Scalar: exps per chunk (pairs are strided over two psum
    # banks; bias is the per-bank constant shift, identical across the
    # pair's mt in the always-taken fast path).
    scale = 1.0 / (SUPCON_T * FP8_SCALE * FP8_SCALE)
    nc.scalar.wait_ge(shift_sem, 16)

    def chunk_ap(sis, nt):
        bk = SG_ACC[sis[0]]
        if len(sis) == 2:
            return acc_all[:, bk:bk + 2, nt * 512:(nt + 1) * 512]
        return acc_all[:, bk, nt * 512:(nt + 1) * 512]

    for ci, (sis, nt, col0, owner) in enumerate(CHUNKS):
        b, mt, a = SGS[sis[0]]
        bias = shift_t[:, 0, a, b:b + 1]
        nc.scalar.wait_ge(s_mm, ci + 1)
        if owner == 'a':
            nc.scalar.activation(
                out=chunk_ap(sis, nt), in_=chunk_ap(sis, nt),
                func=mybir.ActivationFunctionType.Exp,
                bias=bias, scale=scale,
                accum_out=rs[:, col0:col0 + 1]).then_inc(s_acc, 1)
        else:
            nc.scalar.activation(
                out=chunk_ap(sis, nt), in_=chunk_ap(sis, nt),
                func=mybir.ActivationFunctionType.Exp,
                bias=bias, scale=scale).then_inc(s_exp, 1)

    # ---- Vector: row reduces per chunk (pair reduce keeps the bank
    # dim: [128, 2, 512] -X-> [128, 2]).
    for ci, (sis, nt, col0, owner) in enumerate(CHUNKS):
        if owner != 'v':
            continue
        nc.vector.wait_ge(s_exp, ci + 1)
        nc.vector.tensor_reduce(out=rs[:, col0:col0 + len(sis)],
                                in_=chunk_ap(sis, nt),
                                axis=mybir.AxisListType.X,
                                op=mybir.AluOpType.add).then_inc(s_redv, 1)

    # ---- Sync: output DMA once every rs column is written.
    nv = sum(1 for c in CHUNKS if c[3] == 'v')
    na = sum(1 for c in CHUNKS if c[3] == 'a')
    nc.sync.wait_ge(s_redv, nv)
    nc.sync.wait_ge(s_acc, na)
    nc.sync.dma_start(out=res, in_=rs).then_inc(s_fin, 16)

    # No explicit drains/barrier: the neuronxcc custom-kernel lowering
    # appends its own all-engine barrier + drains + sem sweep.

    nc.compile()
    return nc


def get_nc():
    if "nc" not in _NC_CACHE:
        _NC_CACHE["nc"] = _build_nc()
    return _NC_CACHE["nc"]


def _l2norm(x):
    n = np.linalg.norm(x, axis=-1, keepdims=True)
    return x / np.maximum(n, 1e-12)


def _gather_positives(feats_b, lab_a, mlab_b):
    """G[i] = sum of bank rows whose prototype label == lab_a[i]."""
    G = np.zeros((B, D), np.float32)
    if np.unique(mlab_b).size == mlab_b.size:
        inv = np.full(1 << 14, -1, np.int64)
        inv[mlab_b] = np.arange(mlab_b.size)
        idx = inv[np.clip(lab_a, 0, (1 << 14) - 1)]
        valid = idx >= 0
        G[valid] = feats_b[idx[valid]]
    else:
        by_label = np.zeros((1 << 14, D), np.float32)
        np.add.at(by_label, mlab_b, feats_b)
        G[:] = by_label[np.clip(lab_a, 0, (1 << 14) - 1)]
    return G


def make_in_maps(inputs_rgb, inputs_ir, targets_rgb, targets_ir,
                 features_rgb, features_ir,
                 prototype_labels_rgb, prototype_labels_ir):
    x = [_l2norm(np.asarray(inputs_rgb, np.float32)),
         _l2norm(np.asarray(inputs_ir, np.float32))]
    feats = [np.asarray(features_rgb, np.float32),
             np.asarray(features_ir, np.float32)]
    lab = [np.asarray(targets_rgb).astype(np.int64),
           np.asarray(targets_ir).astype(np.int64)]
    mlab = [np.asarray(prototype_labels_rgb).astype(np.int64),
            np.asarray(prototype_labels_ir).astype(np.int64)]

    # xT[a] = [128, KT, B]: x[a].T tiled over kt.
    xT = np.empty([2, 128, KT, B], np.float32)
    for a in range(2):
        xT[a] = (x[a].T.reshape(KT, 128, B) * FP8_SCALE).transpose(1, 0, 2)
    xT = np.ascontiguousarray(xT).astype(FP8_NP)

    # Per-bank constant logsumexp shift: bank_max[b] >= any |x.m| since
    # x is unit-norm (Cauchy-Schwarz), so exp never overflows.  The
    # device kernel's merged pair-exps rely on this being constant
    # across mt within a (a, b) combo.
    bank_max = [float(np.sqrt((feats[b] ** 2).sum(axis=1).max()))
                for b in range(2)]
    shift = np.empty((B, 2, 2), np.float64)                   # [i, a, b]
    for b in range(2):
        shift[:, :, b] = bank_max[b] / SUPCON_T
    nshift = np.ascontiguousarray(
        (-shift).reshape(MT, 128, 2, 2).transpose(1, 0, 2, 3)).astype(np.float32)

    # Host-side positives: pos[a][b][i] = x[a][i] . G_ab[i].
    pos = np.empty((2, 2, B), np.float64)
    for a in range(2):
        for b in range(2):
            G = _gather_positives(feats[b], lab[a], mlab[b])
            pos[a, b] = (x[a].astype(np.float64) *
                         G.astype(np.float64)).sum(axis=1)

    in_maps = []
    for c in range(NCORES):
        memB = np.empty([2, 2, 128, KT, 512], FP8_NP)
        for b in range(2):
            for nt in range(2):
                b_rows = feats[b][c * NS + nt * 512:c * NS + (nt + 1) * 512, :]
                memB[b, nt] = (b_rows.T * FP8_SCALE).reshape(
                    KT, 128, 512).transpose(1, 0, 2).astype(FP8_NP)
        in_maps.append({
            "xT": xT,
            "memB": memB,
            "nshift": nshift,
        })
    return in_maps, (shift, pos)


def combine(results, aux, targets_rgb, targets_ir,
            prototype_labels_rgb, prototype_labels_ir):
    shift, pos = aux
    rs = np.stack([np.asarray(r["res"], np.float64) for r in results])
    rs_sum = rs.sum(axis=0)                                    # [128, NOUT]
    sumexp = np.zeros((B, 4), np.float64)
    for sis, nt, col0, owner in CHUNKS:
        for j, si in enumerate(sis):
            b, mt, a = SGS[si]
            c = a * 2 + b
            sumexp[mt * 128:(mt + 1) * 128, c] += rs_sum[:, col0 + j]

    lab = [np.asarray(targets_rgb).astype(np.int64),
           np.asarray(targets_ir).astype(np.int64)]
    mlab = [np.asarray(prototype_labels_rgb).astype(np.int64),
            np.asarray(prototype_labels_ir).astype(np.int64)]

    losses = np.zeros(4, np.float64)
    for a in range(2):
        for b in range(2):
            c = a * 2 + b
            lse = shift[:, a, b] + np.log(sumexp[:, c])
            cnt = np.bincount(mlab[b], minlength=1 << 14)[
                np.clip(lab[a], 0, (1 << 14) - 1)].astype(np.float64)
            mlpp = (pos[a, b] / SUPCON_T - cnt * lse) / np.maximum(cnt, 1.0)
            losses[c] = -mlpp.mean()

    loss_contr = losses[0] + losses[3]        # (rgb,rgb) + (ir,ir)
    loss_cross = losses[1] + losses[2]        # (rgb,ir)  + (ir,rgb)
    return np.asarray([loss_contr, loss_cross], np.float32)


def run_device(in_maps, **kwargs):
    return run_bass_kernel_spmd(get_nc(), in_maps,
                                core_ids=list(range(NCORES)), **kwargs)


def kernel(inputs_rgb, inputs_ir, targets_rgb, targets_ir,
           features_rgb, features_ir,
           prototype_labels_rgb, prototype_labels_ir):
    in_maps, aux = make_in_maps(inputs_rgb, inputs_ir, targets_rgb,
                                targets_ir, features_rgb, features_ir,
                                prototype_labels_rgb, prototype_labels_ir)
    results = run_device(in_maps).results
    return combine(results, aux, targets_rgb, targets_ir,
                   prototype_labels_rgb, prototype_labels_ir)


# revision 36
# speedup vs baseline: 1.0725x; 1.0725x over previous
"""SupCon cluster-memory loss kernel for 8 TRN2 NeuronCores — raw bass.

Math (per core, N-shard of 1024 bank rows x 4 (anchor, bank) combos):
  sumexp[i] = sum_j exp((x_a . mem_b_j)/T - shift_b)
via fp8 DoubleRow matmuls + ScalarE Exp + VectorE/GpSimd row-sums.
The positives term is host-side index bookkeeping (no device work).

Schedule (vs the whole-block baseline, ~28.1us -> ~27.5us max-core):
- The stream-critical front (x0 + bank-1 slices) is kp-sliced (<=131KB)
  across both HWDGE rings in consumption order: probe-measured, a small
  transfer's completion semaphore lands with its data, while the 393KB
  whole-block completions can lag data by ~2.3us when more transfers
  are queued behind.  The two bank-0 blocks ship whole at the scalar
  ring's tail (their laggier sems still land 2-4us before use), and
  x1/shift ride the gpsimd SWDGE ring.
- PE stream is kp-quad-major: each bank slice feeds four DoubleRow
  matmuls (0.86us of PE work) matching the ~2-queue fill rate, so the
  stream runs nearly gaplessly from ~10.3us (HAM full-clock) onward.
  20 back-to-back warmups keep the PE busy through the clock ramp; too
  few (or an idle gap) leaves the stream at 1.2GHz for milliseconds of
  matmuls (measured v3/v10 regressions).
- Exp chunks: the b1 sgs exp as strided [128,2,512] pair-ACTIVATEs
  (valid: per-bank-constant shift), the b0 sgs as singles in sg-serial
  stream order so Scalar's b0 crunch starts early; the last two chunks
  use the fused ACT row-accumulator so the post-stream tail is just
  exp+read+output-DMA (~1.6us).
- No trailing drains or barrier: the neuronxcc custom-kernel lowering
  appends its own all-engine barrier + drains + a fixed ~7us sweep
  zeroing sems 3..255 on every execution (the dominant fixed tax).
"""

import ml_dtypes
import numpy as np

import concourse.bacc as bacc
import concourse.mybir as mybir
from concourse.bass_utils import run_bass_kernel_spmd

BF16_NP = ml_dtypes.bfloat16

B = 256          # anchor batch per modality
N = 8192         # memory bank rows
D = 768          # feature dim
NCORES = 8
NS = N // NCORES     # 1024 bank rows per core
KT = D // 128        # 6 contraction tiles
MT = B // 128        # 2 anchor partition tiles
SUPCON_T = 0.07

F32 = mybir.dt.float32
FP8 = mybir.dt.float8e4
FP8_NP = ml_dtypes.float8_e4m3
FP8_SCALE = 16.0

NWU = 20         # warmup matmuls: ~4.3us of continuous PE work so the
                 # HAM clock ramp reaches 2.4GHz before the real stream
NOUT = 16        # rs cols: one per sg-half (see CHUNKS)

# Supergroups (b, mt, a); psum bank = si % 4 (b=1 sgs then b=0 reuse).
SGS = [(1, 0, 0), (1, 1, 0), (1, 0, 1), (1, 1, 1),
       (0, 0, 0), (0, 1, 0), (0, 0, 1), (0, 1, 1)]
SG_ACC = [0, 1, 2, 3, 0, 1, 2, 3]

# Exp chunks in s_mm order: (si_list, nt, rs_col_start, owner).
# b1 chunks are strided [128, 2, 512] pair-ACTIVATEs over adjacent psum
# banks — valid because the logsumexp shift is a per-bank constant
# (normalized prototypes; see make_in_maps).  The b0 chunks are singles
# (sg-serial stream order) so the Scalar engine's b0 exp crunch starts
# as early as possible; the last two use the fused ACT row-accumulator.
CHUNKS = [([0, 1], 0, 0, 'v'), ([2, 3], 0, 2, 'v'),
          ([0, 1], 1, 4, 'v'), ([2, 3], 1, 6, 'v'),
          ([4], 0, 8, 'v'), ([5], 0, 9, 'v'),
          ([6], 0, 10, 'v'), ([7], 0, 11, 'v'),
          ([4, 5], 1, 12, 'v'),
          ([6], 1, 14, 'a'), ([7], 1, 15, 'a')]

_NC_CACHE = {}


def _build_nc():
    nc = bacc.Bacc("TRN2", target_bir_lowering=False, debug=False,
                   num_devices=NCORES)

    # xT split by anchor half a: [2][128, KT, B] fp8.
    xT = nc.dram_tensor("xT", [2, 128, KT, B], FP8, kind="ExternalInput").ap()
    # memB[b][nt] = [128, KT, 512] fp8 column block of bank b.
    memB = nc.dram_tensor("memB", [2, 2, 128, KT, 512], FP8,
                          kind="ExternalInput").ap()
    nshift_h = nc.dram_tensor("nshift", [128, MT, 2, 2], F32,
                              kind="ExternalInput").ap()
    res = nc.dram_tensor("res", [128, NOUT], F32, kind="ExternalOutput").ap()

    x_sb = [nc.alloc_sbuf_tensor(f"x{a}", [128, KT, B], FP8).ap()
            for a in range(2)]
    blk = {(b, nt): nc.alloc_sbuf_tensor(f"m{b}{nt}", [128, KT, 512], FP8).ap()
           for b in range(2) for nt in range(2)}
    shift_t = nc.alloc_sbuf_tensor("shift", [128, MT, 2, 2], F32).ap()
    rs = nc.alloc_sbuf_tensor("rs", [128, NOUT], F32).ap()
    wu_w = nc.alloc_sbuf_tensor("wu_w", [128, 128], FP8).ap()
    wu_r = nc.alloc_sbuf_tensor("wu_r", [128, 256], FP8).ap()
    acc_all = nc.alloc_psum_tensor("acc", [128, 4, 1024], F32).ap()

    def sem(name):
        return nc.alloc_semaphore(name)

    # ---- DMA slice plan.  Each entry: (engine, kind, args) in queue
    # order; kind 'b' = bank slice (b, nt, kp), 'x' = x slice (a, kp),
    # 'xw' = x whole-remainder, 's' = shift.
    # Only the b1/x0 front needs tight per-slice completion sems; it is
    # kp-sliced across both HWDGE rings in consumption order.  The b0
    # banks ship as whole 393KB blocks ('B'): better queue throughput,
    # and their (laggier) completions still land 2-4us before use.
    # x1/shift ride the gpsimd SWDGE.
    q_scalar = [('x', 0, 0), ('b', 1, 0, 0), ('b', 1, 0, 2), ('B', 0, 0),
                ('B', 0, 1)]
    q_sync = [('x', 0, 1), ('b', 1, 0, 1), ('x', 0, 2), ('b', 1, 1, 0),
              ('b', 1, 1, 1), ('b', 1, 1, 2)]
    q_gp = [('s',), ('x', 1, 0), ('x', 1, 1), ('x', 1, 2)]

    bank_sem = {}
    x_sem = {}
    shift_sem = None

    def issue(eng, plan):
        nonlocal shift_sem
        for item in plan:
            if item[0] == 'b':
                _, b, nt, kp = item
                s = sem(f"d_b{b}{nt}{kp}")
                eng.dma_start(
                    out=blk[b, nt][:, 2 * kp:2 * kp + 2],
                    in_=memB[b, nt][:, 2 * kp:2 * kp + 2]).then_inc(s, 16)
                bank_sem[(b, nt, kp)] = s
            elif item[0] == 'B':
                _, b, nt = item
                s = sem(f"d_B{b}{nt}")
                eng.dma_start(out=blk[b, nt],
                              in_=memB[b, nt]).then_inc(s, 16)
                for kp in range(KT // 2):
                    bank_sem[(b, nt, kp)] = s
            elif item[0] == 'x':
                _, a, kp = item
                s = sem(f"d_x{a}{kp}")
                eng.dma_start(
                    out=x_sb[a][:, 2 * kp:2 * kp + 2],
                    in_=xT[a][:, 2 * kp:2 * kp + 2]).then_inc(s, 16)
                x_sem[(a, kp)] = s
            else:
                s = sem("d_shift")
                eng.dma_start(out=shift_t, in_=nshift_h).then_inc(s, 16)
                shift_sem = s

    issue(nc.scalar, q_scalar)
    issue(nc.sync, q_sync)
    issue(nc.gpsimd, q_gp)

    s_mm = sem("s_mm")      # chunk accumulation done (Tensor), CHUNKS order
    s_exp = sem("s_exp")    # chunk exp done (Scalar)
    s_redv = sem("s_redv")  # Vector reduces done
    s_redg = sem("s_redg")  # GpSimd reduces done
    s_acc = sem("s_acc")    # final accum chunk retired (Scalar)
    s_fin = sem("s_fin")    # output DMA

    # ---- Tensor: warmups then the 48-matmul stream, kp-quad-major:
    # each bank slice feeds four matmuls (one per sg of the phase), so
    # ~0.86us of PE work per 131KB slice matches the 2-queue fill rate.
    for _ in range(NWU):
        nc.tensor.matmul(acc_all[:, 0, 0:256], wu_w, wu_r,
                         start=True, stop=True)

    hi = {}

    def twait(s, v):
        if hi.get(s.num, 0) < v:
            hi[s.num] = v
            nc.tensor.wait_ge(s, v)

    def task(si, nt, kp):
        b, mt, a = SGS[si]
        twait(bank_sem[(b, nt, kp)], 16)
        twait(x_sem[(a, kp)], 16)
        return nc.tensor.matmul(
            acc_all[:, SG_ACC[si], nt * 512:(nt + 1) * 512],
            x_sb[a][:, 2 * kp:2 * kp + 2, mt * 128:(mt + 1) * 128],
            blk[b, nt][:, 2 * kp:2 * kp + 2],
            start=(kp == 0), stop=(kp == KT // 2 - 1),
            perf_mode=mybir.MatmulPerfMode.DoubleRow)

    KL = KT // 2
    # b1 phases: kp-quad-major (supply-matched to the sliced fill).
    # Chunk sems (s_mm) fire on the pair's later sg at kp2.
    for nt in range(2):
        for kp in range(KL):
            for si in range(4):
                mm = task(si, nt, kp)
                if kp == KL - 1 and si in (1, 3):
                    mm.then_inc(s_mm, 1)
    # b0 phases: whole blocks arrive by ~13us, so order sg-serial for the
    # earliest possible per-sg chunk completions.  WAR: sg si reuses the
    # psum region of sg si-4, freed by that chunk's Vector reduce.
    for nt in range(2):
        for si in range(4, 8):
            # chunk index of the donor (b1) chunk in s_redv counting:
            # nt0: c1 (banks 0-1) / c2 (banks 2-3); nt1: c3 / c4.
            twait(s_redv, 2 * nt + 1 + (si >= 6))
            for kp in range(KL):
                mm = task(si, nt, kp)
            # b0-nt1: si4/si5 share a pair chunk ending at si5.
            if not (nt == 1 and si == 4):
                mm.then_inc(s_mm, 1)

    # ---- Scalar: exps per chunk (pairs are strided over two psum
    # banks; bias is the per-bank constant shift, identical across the
    # pair's mt in the always-taken fast path).
    scale = 1.0 / (SUPCON_T * FP8_SCALE * FP8_SCALE)
    nc.scalar.wait_ge(shift_sem, 16)

    def chunk_ap(sis, nt):
        bk = SG_ACC[sis[0]]
        if len(sis) == 2:
            return acc_all[:, bk:bk + 2, nt * 512:(nt + 1) * 512]
        return acc_all[:, bk, nt * 512:(nt + 1) * 512]

    for ci, (sis, nt, col0, owner) in enumerate(CHUNKS):
        b, mt, a = SGS[sis[0]]
        bias = shift_t[:, 0, a, b:b + 1]
        nc.scalar.wait_ge(s_mm, ci + 1)
        if owner == 'a':
            nc.scalar.activation(
                out=chunk_ap(sis, nt), in_=chunk_ap(sis, nt),
                func=mybir.ActivationFunctionType.Exp,
                bias=bias, scale=scale,
                accum_out=rs[:, col0:col0 + 1]).then_inc(s_acc, 1)
        else:
            nc.scalar.activation(
                out=chunk_ap(sis, nt), in_=chunk_ap(sis, nt),
                func=mybir.ActivationFunctionType.Exp,
                bias=bias, scale=scale).then_inc(s_exp, 1)

    # ---- Vector: row reduces per chunk (pair reduce keeps the bank
    # dim: [128, 2, 512] -X-> [128, 2]).
    for ci, (sis, nt, col0, owner) in enumerate(CHUNKS):
        if owner != 'v':
            continue
        nc.vector.wait_ge(s_exp, ci + 1)
        nc.vector.tensor_reduce(out=rs[:, col0:col0 + len(sis)],
                                in_=chunk_ap(sis, nt),
                                axis=mybir.AxisListType.X,
                                op=mybir.AluOpType.add).then_inc(s_redv, 1)

    # ---- Sync: output DMA once every rs column is written.
    nv = sum(1 for c in CHUNKS if c[3] == 'v')
    na = sum(1 for c in CHUNKS if c[3] == 'a')
    nc.sync.wait_ge(s_redv, nv)
    nc.sync.wait_ge(s_acc, na)
    nc.sync.dma_start(out=res, in_=rs).then_inc(s_fin, 16)

    # No explicit drains/barrier: the neuronxcc custom-kernel lowering
    # appends its own all-engine barrier + drains + sem sweep.

    nc.compile()
    return nc


def get_nc():
    if "nc" not in _NC_CACHE:
        _NC_CACHE["nc"] = _build_nc()
    return _NC_CACHE["nc"]


def _l2norm(x):
    n = np.linalg.norm(x, axis=-1, keepdims=True)
    return x / np.maximum(n, 1e-12)


def _gather_positives(feats_b, lab_a, mlab_b):
    """G[i] = sum of bank rows whose prototype label == lab_a[i]."""
    G = np.zeros((B, D), np.float32)
    if np.unique(mlab_b).size == mlab_b.size:
        inv = np.full(1 << 14, -1, np.int64)
        inv[mlab_b] = np.arange(mlab_b.size)
        idx = inv[np.clip(lab_a, 0, (1 << 14) - 1)]
        valid = idx >= 0
        G[valid] = feats_b[idx[valid]]
    else:
        by_label = np.zeros((1 << 14, D), np.float32)
        np.add.at(by_label, mlab_b, feats_b)
        G[:] = by_label[np.clip(lab_a, 0, (1 << 14) - 1)]
    return G


def make_in_maps(inputs_rgb, inputs_ir, targets_rgb, targets_ir,
                 features_rgb, features_ir,
                 prototype_labels_rgb, prototype_labels_ir):
    x = [_l2norm(np.asarray(inputs_rgb, np.float32)),
         _l2norm(np.asarray(inputs_ir, np.float32))]
    feats = [np.asarray(features_rgb, np.float32),
             np.asarray(features_ir, np.float32)]
    lab = [np.asarray(targets_rgb).astype(np.int64),
           np.asarray(targets_ir).astype(np.int64)]
    mlab = [np.asarray(prototype_labels_rgb).astype(np.int64),
            np.asarray(prototype_labels_ir).astype(np.int64)]

    # xT[a] = [128, KT, B]: x[a].T tiled over kt.
    xT = np.empty([2, 128, KT, B], np.float32)
    for a in range(2):
        xT[a] = (x[a].T.reshape(KT, 128, B) * FP8_SCALE).transpose(1, 0, 2)
    xT = np.ascontiguousarray(xT).astype(FP8_NP)

    # Per-bank constant logsumexp shift: bank_max[b] >= any |x.m| since
    # x is unit-norm (Cauchy-Schwarz), so exp never overflows.  The
    # device kernel's merged pair-exps rely on this being constant
    # across mt within a (a, b) combo.
    bank_max = [float(np.sqrt((feats[b] ** 2).sum(axis=1).max()))
                for b in range(2)]
    shift = np.empty((B, 2, 2), np.float64)                   # [i, a, b]
    for b in range(2):
        shift[:, :, b] = bank_max[b] / SUPCON_T
    nshift = np.ascontiguousarray(
        (-shift).reshape(MT, 128, 2, 2).transpose(1, 0, 2, 3)).astype(np.float32)

    # Host-side positives: pos[a][b][i] = x[a][i] . G_ab[i].
    pos = np.empty((2, 2, B), np.float64)
    for a in range(2):
        for b in range(2):
            G = _gather_positives(feats[b], lab[a], mlab[b])
            pos[a, b] = (x[a].astype(np.float64) *
                         G.astype(np.float64)).sum(axis=1)

    in_maps = []
    for c in range(NCORES):
        memB = np.empty([2, 2, 128, KT, 512], FP8_NP)
        for b in range(2):
            for nt in range(2):
                b_rows = feats[b][c * NS + nt * 512:c * NS + (nt + 1) * 512, :]
                memB[b, nt] = (b_rows.T * FP8_SCALE).reshape(
                    KT, 128, 512).transpose(1, 0, 2).astype(FP8_NP)
        in_maps.append({
            "xT": xT,
            "memB": memB,
            "nshift": nshift,
        })
    return in_maps, (shift, pos)


def combine(results, aux, targets_rgb, targets_ir,
            prototype_labels_rgb, prototype_labels_ir):
    shift, pos = aux
    rs = np.stack([np.asarray(r["res"], np.float64) for r in results])
    rs_sum = rs.sum(axis=0)                                    # [128, NOUT]
    sumexp = np.zeros((B, 4), np.float64)
    for sis, nt, col0, owner in CHUNKS:
        for j, si in enumerate(sis):
            b, mt, a = SGS[si]
            c = a * 2 + b
            sumexp[mt * 128:(mt + 1) * 128, c] += rs_sum[:, col0 + j]

    lab = [np.asarray(targets_rgb).astype(np.int64),
           np.asarray(targets_ir).astype(np.int64)]
    mlab = [np.asarray(prototype_labels_rgb).astype(np.int64),
            np.asarray(prototype_labels_ir).astype(np.int64)]

    losses = np.zeros(4, np.float64)
    for a in range(2):
        for b in range(2):
            c = a * 2 + b
            lse = shift[:, a, b] + np.log(sumexp[:, c])
            cnt = np.bincount(mlab[b], minlength=1 << 14)[
                np.clip(lab[a], 0, (1 << 14) - 1)].astype(np.float64)
            mlpp = (pos[a, b] / SUPCON_T - cnt * lse) / np.maximum(cnt, 1.0)
            losses[c] = -mlpp.mean()

    loss_contr = losses[0] + losses[3]        # (rgb,rgb) + (ir,ir)
    loss_cross = losses[1] + losses[2]        # (rgb,ir)  + (ir,rgb)
    return np.asarray([loss_contr, loss_cross], np.float32)


def run_device(in_maps, **kwargs):
    return run_bass_kernel_spmd(get_nc(), in_maps,
                                core_ids=list(range(NCORES)), **kwargs)


def kernel(inputs_rgb, inputs_ir, targets_rgb, targets_ir,
           features_rgb, features_ir,
           prototype_labels_rgb, prototype_labels_ir):
    in_maps, aux = make_in_maps(inputs_rgb, inputs_ir, targets_rgb,
                                targets_ir, features_rgb, features_ir,
                                prototype_labels_rgb, prototype_labels_ir)
    results = run_device(in_maps).results
    return combine(results, aux, targets_rgb, targets_ir,
                   prototype_labels_rgb, prototype_labels_ir)
